# revision 30
# baseline (speedup 1.0000x reference)
"""Trainium2 Bass kernel for shifted sparse attention (nn_Attention_74672301408506).

Math (reference):
    q = x @ W.T ; k = x_key @ W.T ; att = softmax(q k^T)
    out[i, v] = sum_s w_s * sum_j att[i-2s, j] * x_value[j+2s, v]

Device algorithm (8 cores, query rows sharded, 512 rows per core, no halo):
    per core, query rows [r0, r0+512):
      qT[h, i], kT[h, j]     (float32r = tf32 matmuls, full-rate at N>=256)
      S^T[j, i] = kT^T q     (scores transposed: keys on partitions)
      E = exp(S - 100)       (bf16; softmax is shift-invariant)
      Ru[i, :] = E^T @ [V_0|V_1|V_2|V_3|ones]   (ones col = softmax denominator)
      R = Ru[:, :320] / Ru[:, 320]
      C[t, v] = sum_s w_s R[t - 2s, 80s + v]  for t in [0, 518)  (banded matmuls)
    C rows [0, 512) are complete except the first 6 rows, which miss the
    neighbor's contribution; rows [512, 518) are exactly that contribution for
    the next core. The host adds the 6-row overlaps when unsharding (exact).
"""

import os
import sys
import types

import numpy as np
import ml_dtypes

T = 4096
Q = 256
H = 256
NV = 80
NS = 4
STEP = 2
NCORES = 8
M = T // NCORES            # 512 rows per core
CSUB = 100.0               # global score shift before exp
P = 128
NJ = T // P                # 32 key tiles
NF = Q // P                # 2 feature tiles
NH = H // P                # 2 hidden tiles
NMAIN = M // P             # 4 i-chunks of 128
NVC = NS * NV + 1          # value width incl ones column
TAIL = (NS - 1) * STEP     # 6 overlap rows between neighboring cores


def _install_axon_ntff_hook():
    """bass_utils' trace path imports antenv.axon_hooks, which the agent image
    lacks; shim it and register the ctypes-based NTFF profiler hook."""
    if "antenv.axon_hooks" in sys.modules:
        return
    try:
        import antenv
    except ImportError:
        return
    mod = types.ModuleType("antenv.axon_hooks")
    mod._hook = None
    mod.set_axon_ntff_profile_hook = lambda h: setattr(mod, "_hook", h)
    mod.get_axon_ntff_profile_hook = lambda: mod._hook
    sys.modules["antenv.axon_hooks"] = mod
    antenv.axon_hooks = mod
    try:
        from trn_agent_boot import trn_boot

        so_path = "/opt/axon/libaxon_pjrt.so"
        if os.path.exists(so_path):
            mod.set_axon_ntff_profile_hook(trn_boot._ntff_profile_via_ctypes(so_path))
    except Exception:
        pass


_NC_CACHE = {}
LAST_RESULT = None


def _build_nc():
    import concourse.mybir as mybir
    import concourse.tile as tile
    from concourse import bacc

    f32 = mybir.dt.float32
    f32r = mybir.dt.float32r
    bf16 = mybir.dt.bfloat16
    Exp = mybir.ActivationFunctionType.Exp

    nc = bacc.Bacc(None, target_bir_lowering=False)

    xT_d = nc.dram_tensor("xT", [Q, M], f32r, kind="ExternalInput")
    xkT_d = nc.dram_tensor("xkT", [Q, T], f32r, kind="ExternalInput")
    # G = W^T W (host-side): scores = x_key G x^T, so kT is never materialized
    g_d = nc.dram_tensor("gmat", [Q, Q], f32r, kind="ExternalInput")
    # pre-tiled on host: row p holds tile-row p of every j-tile (big DMA descs)
    vc_d = nc.dram_tensor("vcomb", [P, NJ * NVC], bf16, kind="ExternalInput")
    aux_d = nc.dram_tensor("aux", [P, 8 * P + NS * TAIL], bf16, kind="ExternalInput")
    out_d = nc.dram_tensor("out", [NMAIN, P, NV], f32, kind="ExternalOutput")
    outt_d = nc.dram_tensor("outt", [TAIL, NV], f32, kind="ExternalOutput")

    with tile.TileContext(nc) as tc:
        with (
            tc.tile_pool(name="consts", bufs=1) as consts,
            tc.tile_pool(name="io", bufs=1) as io,
            tc.tile_pool(name="store", bufs=1) as store,
            tc.tile_pool(name="small", bufs=6) as small,
            tc.tile_pool(name="psA", bufs=4, space="PSUM") as psA,
            tc.tile_pool(name="psR", bufs=4, space="PSUM") as psR,
        ):
            # ---- PE warmup: dummy matmuls while input DMAs stream, so HAM
            # reaches K=8/8 (2.4 GHz) before real work ----
            wu = consts.tile([P, 512], bf16, name="wu")
            nc.vector.memset(wu, 0.0)
            wups = psA.tile([P, 512], f32, name="wups", tag="ps")
            for i in range(9):
                nc.tensor.matmul(wups, wu[:, 0:P], wu, start=True, stop=True)

            # ---- inputs; spread across HWDGE rings (sync+scalar) and SWDGE ----
            gt = []
            for f in range(NF):
                t = consts.tile([P, Q], f32r, name=f"gt{f}", tag=f"gt{f}")
                eng = nc.sync if f == 0 else nc.scalar
                eng.dma_start(out=t, in_=g_d[P * f : P * (f + 1), :])
                gt.append(t)
            xt = []
            for f in range(NF):
                t = consts.tile([P, M], f32r, name=f"xt{f}", tag=f"xt{f}")
                eng = nc.sync if f == 0 else nc.scalar
                eng.dma_start(out=t, in_=xT_d[P * f : P * (f + 1), :])
                xt.append(t)
            NE = 4  # quarter-chunks of the key stream, [128, 1024] each
            xkh = []
            for f in range(NF):
                pieces = []
                for qq in range(NE):
                    t = io.tile(
                        [P, T // NE], f32r, name=f"xkh{f}_{qq}", tag=f"xk{f}_{qq}"
                    )
                    eng = nc.sync if f == 0 else nc.scalar
                    eng.dma_start(
                        out=t,
                        in_=xkT_d[
                            P * f : P * (f + 1), (T // NE) * qq : (T // NE) * (qq + 1)
                        ],
                    )
                    pieces.append(t)
                xkh.append(pieces)
            # vcomb in 4 group-DMAs: group 0 early on the gpsimd ring; groups
            # 1-3 ride the sync/scalar HWDGE rings BEHIND the xk quarters (ring
            # FIFO order keeps them off the critical early HBM window)
            vcall = store.tile([P, NJ * NVC], bf16, name="vcall", tag="vc")
            GW = 8 * NVC
            for g, eng in [(0, nc.gpsimd), (1, nc.sync), (2, nc.scalar), (3, nc.sync)]:
                eng.dma_start(
                    out=vcall[:, GW * g : GW * (g + 1)],
                    in_=vc_d[:, GW * g : GW * (g + 1)],
                )
            vc = [vcall[:, NVC * j : NVC * (j + 1)] for j in range(NJ)]
            aux = consts.tile([P, 8 * P + NS * TAIL], bf16, name="aux")
            nc.gpsimd.dma_start(out=aux, in_=aux_d[:, :])
            bias_t = consts.tile([P, 1], f32, name="bias_t")
            nc.vector.memset(bias_t, -CSUB)

            # ---- zT = G @ x^T  (the q-side projection through G) ----
            zt = []
            for h in range(NH):
                ps = psA.tile([P, M], f32, name=f"zps{h}", tag="ps")
                for f in range(NF):
                    nc.tensor.matmul(
                        ps,
                        gt[f][:, P * h : P * (h + 1)],
                        xt[f],
                        start=(f == 0),
                        stop=(f == NF - 1),
                    )
                t = store.tile([P, M], f32r, name=f"zt{h}", tag=f"zt{h}")
                if h == 0:
                    nc.vector.tensor_copy(t, ps)
                else:
                    nc.scalar.copy(t, ps)
                zt.append(t)

            # ---- pipeline: S^T(j) = x_keyT-tile^T @ zT directly from the DMA
            # quarters; exp; Ru lagging DELAY j-tiles ----
            ru = []
            for c in range(NMAIN):
                ru.append(psR.tile([P, NVC], f32, name=f"ru{c}", tag="ru"))
            elist = []
            DELAY = 2

            def ru_step(j):
                for c in range(NMAIN):
                    nc.tensor.matmul(
                        ru[c],
                        elist[j][:, P * c : P * (c + 1)],
                        vc[j],
                        start=(j == 0),
                        stop=(j == NJ - 1),
                    )

            for j in range(NJ):
                qq, jq = divmod(j, 8)
                ps = psA.tile([P, M], f32, name=f"sps{j}", tag="ps")
                for f in range(NF):
                    nc.tensor.matmul(
                        ps,
                        xkh[f][qq][:, P * jq : P * (jq + 1)],
                        zt[f],
                        start=(f == 0),
                        stop=(f == NF - 1),
                    )
                ej = store.tile([P, M], bf16, name=f"e{j}", tag="E", bufs=NJ)
                nc.scalar.activation(ej, ps, Exp, bias=bias_t)
                elist.append(ej)
                if j >= DELAY:
                    ru_step(j - DELAY)
            for j in range(NJ - DELAY, NJ):
                ru_step(j)

            # ---- normalize + combine, chunk by chunk ----
            # aux layout (bf16), all [128, 128] banded matrices with w_s baked:
            #   A1 = aux[:, 128s:128(s+1)]        k == p - 2s       (own chunk)
            #   A2 = aux[:, 512+128s:512+128(s+1)] k == 128 + p - 2s (prev chunk)
            #   A3 = aux[:, 1024+6s:1024+6(s+1)]  k == 128 + t' - 2s (tail rows)
            rch = []
            for c in range(NMAIN):
                rec = small.tile([P, 1], f32, name=f"rec{c}", tag="rec")
                nc.vector.reciprocal(rec, ru[c][:, NS * NV : NVC])
                t = store.tile([P, NS * NV], bf16, name=f"r{c}", tag=f"r{c}")
                half = NS * NV // 2
                nc.vector.tensor_scalar_mul(t[:, 0:half], ru[c][:, 0:half], rec)
                nc.scalar.mul(t[:, half:], ru[c][:, half : NS * NV], rec)
                rch.append(t)

            oall = small.tile([P, NMAIN, NV], f32, name="oall", tag="osb")
            for c in range(NMAIN):
                po = psA.tile([P, NV], f32, name=f"po{c}", tag="ps")
                nmm = NS + (NS - 1 if c > 0 else 0)
                i = 0
                for s in range(NS):
                    nc.tensor.matmul(
                        po,
                        aux[:, P * s : P * (s + 1)],
                        rch[c][:, NV * s : NV * (s + 1)],
                        start=(i == 0),
                        stop=(i == nmm - 1),
                    )
                    i += 1
                if c > 0:
                    for s in range(1, NS):
                        nc.tensor.matmul(
                            po,
                            aux[:, 4 * P + P * s : 4 * P + P * (s + 1)],
                            rch[c - 1][:, NV * s : NV * (s + 1)],
                            start=False,
                            stop=(i == nmm - 1),
                        )
                        i += 1
                nc.vector.tensor_copy(oall[:, c, :], po)
            nc.sync.dma_start(out=out_d.rearrange("c p v -> p c v"), in_=oall)

            # tail rows [512, 518): next core's missing contribution
            pot = psA.tile([TAIL, NV], f32, name="pot", tag="ps")
            for s in range(1, NS):
                nc.tensor.matmul(
                    pot,
                    aux[:, 8 * P + TAIL * s : 8 * P + TAIL * (s + 1)],
                    rch[NMAIN - 1][:, NV * s : NV * (s + 1)],
                    start=(s == 1),
                    stop=(s == NS - 1),
                )
            ot = small.tile([TAIL, NV], f32, name="ot", tag="ot")
            nc.vector.tensor_copy(ot, pot)
            nc.sync.dma_start(out=outt_d[:, :], in_=ot)

    nc.compile()
    return nc


def _get_nc():
    if "nc" not in _NC_CACHE:
        _install_axon_ntff_hook()
        _NC_CACHE["nc"] = _build_nc()
    return _NC_CACHE["nc"]


def _host_prep(x, x_key, x_value, W_qk, w_shift):
    bf = ml_dtypes.bfloat16
    x = np.ascontiguousarray(np.asarray(x, dtype=np.float32))
    x_key = np.ascontiguousarray(np.asarray(x_key, dtype=np.float32))
    x_value = np.ascontiguousarray(np.asarray(x_value, dtype=np.float32))
    W_qk = np.ascontiguousarray(np.asarray(W_qk, dtype=np.float32))
    w_shift = np.asarray(w_shift, dtype=np.float32)

    xkT = np.ascontiguousarray(x_key.T)                      # [Q, T]
    gmat = np.ascontiguousarray(
        (W_qk.astype(np.float64).T @ W_qk.astype(np.float64)).astype(np.float32)
    )                                                        # [Q, Q], symmetric

    vcomb = np.zeros((T, NVC), np.float32)
    for s in range(NS):
        d = STEP * s
        vcomb[: T - d, NV * s : NV * (s + 1)] = x_value[d:, :]
    vcomb[:, NS * NV] = 1.0
    # pre-tile: [T, NVC] -> [NJ, P, NVC] -> [P, NJ*NVC] so each SBUF partition
    # line is one contiguous DMA descriptor
    vcomb = np.ascontiguousarray(
        vcomb.astype(bf).reshape(NJ, P, NVC).transpose(1, 0, 2).reshape(P, NJ * NVC)
    )

    # combine matrices (see aux layout comment in _build_nc)
    aux = np.zeros((P, 8 * P + NS * TAIL), np.float32)
    for s in range(NS):
        w = w_shift[0, s]
        for p in range(P):
            k = p - STEP * s
            if 0 <= k < P:
                aux[k, P * s + p] = w                      # A1
            kk = P + p - STEP * s
            if 0 <= kk < P:
                aux[kk, 4 * P + P * s + p] = w             # A2 (prev chunk)
        if s >= 1:
            for tp in range(TAIL):
                k = P + tp - STEP * s
                if 0 <= k < P:
                    aux[k, 8 * P + TAIL * s + tp] = w      # A3 (tail rows)
    aux = aux.astype(bf)

    in_maps = []
    for d in range(NCORES):
        r0 = d * M
        xT = np.ascontiguousarray(x[r0 : r0 + M].T)          # [Q, M]
        in_maps.append(
            {"xT": xT, "xkT": xkT, "gmat": gmat, "vcomb": vcomb, "aux": aux}
        )
    return in_maps


def kernel(x, x_key, x_value, W_qk, w_shift):
    global LAST_RESULT
    from concourse.bass_utils import run_bass_kernel_spmd

    nc = _get_nc()
    in_maps = _host_prep(x, x_key, x_value, W_qk, w_shift)
    res = run_bass_kernel_spmd(nc, in_maps, core_ids=list(range(NCORES)))
    LAST_RESULT = res
    out = np.concatenate(
        [res.results[d]["out"].reshape(M, NV) for d in range(NCORES)], axis=0
    )
    # add the 6-row cross-core overlap contributions
    for d in range(NCORES - 1):
        out[M * (d + 1) : M * (d + 1) + TAIL] += res.results[d]["outt"]
    return out.astype(np.float32)


# revision 31
# speedup vs baseline: 1.0399x; 1.0399x over previous
"""Trainium2 Bass kernel for shifted sparse attention (nn_Attention_74672301408506).

Math (reference):
    q = x @ W.T ; k = x_key @ W.T ; att = softmax(q k^T)
    out[i, v] = sum_s w_s * sum_j att[i-2s, j] * x_value[j+2s, v]

Device algorithm (8 cores, query rows sharded, 512 rows per core, no halo):
    per core, query rows [r0, r0+512):
      qT[h, i], kT[h, j]     (float32r = tf32 matmuls, full-rate at N>=256)
      S^T[j, i] = kT^T q     (scores transposed: keys on partitions)
      E = exp(S - 100)       (bf16; softmax is shift-invariant)
      Ru[i, :] = E^T @ [V_0|V_1|V_2|V_3|ones]   (ones col = softmax denominator)
      R = Ru[:, :320] / Ru[:, 320]
      C[t, v] = sum_s w_s R[t - 2s, 80s + v]  for t in [0, 518)  (banded matmuls)
    C rows [0, 512) are complete except the first 6 rows, which miss the
    neighbor's contribution; rows [512, 518) are exactly that contribution for
    the next core. The host adds the 6-row overlaps when unsharding (exact).
"""

import os
import sys
import types

import numpy as np
import ml_dtypes

T = 4096
Q = 256
H = 256
NV = 80
NS = 4
STEP = 2
NCORES = 8
M = T // NCORES            # 512 rows per core
CSUB = 100.0               # global score shift before exp
P = 128
NJ = T // P                # 32 key tiles
NF = Q // P                # 2 feature tiles
NH = H // P                # 2 hidden tiles
NMAIN = M // P             # 4 i-chunks of 128
NVC = NS * NV + 1          # value width incl ones column
TAIL = (NS - 1) * STEP     # 6 overlap rows between neighboring cores


def _install_axon_ntff_hook():
    """bass_utils' trace path imports antenv.axon_hooks, which the agent image
    lacks; shim it and register the ctypes-based NTFF profiler hook."""
    if "antenv.axon_hooks" in sys.modules:
        return
    try:
        import antenv
    except ImportError:
        return
    mod = types.ModuleType("antenv.axon_hooks")
    mod._hook = None
    mod.set_axon_ntff_profile_hook = lambda h: setattr(mod, "_hook", h)
    mod.get_axon_ntff_profile_hook = lambda: mod._hook
    sys.modules["antenv.axon_hooks"] = mod
    antenv.axon_hooks = mod
    try:
        from trn_agent_boot import trn_boot

        so_path = "/opt/axon/libaxon_pjrt.so"
        if os.path.exists(so_path):
            mod.set_axon_ntff_profile_hook(trn_boot._ntff_profile_via_ctypes(so_path))
    except Exception:
        pass


_NC_CACHE = {}
LAST_RESULT = None


def _build_nc():
    import concourse.mybir as mybir
    import concourse.tile as tile
    from concourse import bacc

    f32 = mybir.dt.float32
    f32r = mybir.dt.float32r
    bf16 = mybir.dt.bfloat16
    Exp = mybir.ActivationFunctionType.Exp

    nc = bacc.Bacc(None, target_bir_lowering=False)

    xT_d = nc.dram_tensor("xT", [Q, M], f32r, kind="ExternalInput")
    xkT_d = nc.dram_tensor("xkT", [Q, T], f32r, kind="ExternalInput")
    # G = W^T W (host-side): scores = x_key G x^T, so kT is never materialized
    g_d = nc.dram_tensor("gmat", [Q, Q], f32r, kind="ExternalInput")
    # pre-tiled on host: row p holds tile-row p of every j-tile (big DMA descs)
    vc_d = nc.dram_tensor("vcomb", [P, NJ * NVC], bf16, kind="ExternalInput")
    aux_d = nc.dram_tensor("aux", [P, 8 * P + NS * TAIL], bf16, kind="ExternalInput")
    out_d = nc.dram_tensor("out", [NMAIN, P, NV], f32, kind="ExternalOutput")
    outt_d = nc.dram_tensor("outt", [TAIL, NV], f32, kind="ExternalOutput")

    with tile.TileContext(nc) as tc:
        with (
            tc.tile_pool(name="consts", bufs=1) as consts,
            tc.tile_pool(name="io", bufs=1) as io,
            tc.tile_pool(name="store", bufs=1) as store,
            tc.tile_pool(name="small", bufs=6) as small,
            tc.tile_pool(name="psA", bufs=4, space="PSUM") as psA,
            tc.tile_pool(name="psR", bufs=4, space="PSUM") as psR,
        ):
            # ---- PE warmup: dummy matmuls while input DMAs stream, so HAM
            # reaches K=8/8 (2.4 GHz) before real work ----
            wu = consts.tile([P, 512], bf16, name="wu")
            nc.vector.memset(wu, 0.0)
            wups = psA.tile([P, 512], f32, name="wups", tag="ps")
            for i in range(9):
                nc.tensor.matmul(wups, wu[:, 0:P], wu, start=True, stop=True)

            # ---- inputs; spread across HWDGE rings (sync+scalar) and SWDGE ----
            gt = []
            for f in range(NF):
                t = consts.tile([P, Q], f32r, name=f"gt{f}", tag=f"gt{f}")
                eng = nc.sync if f == 0 else nc.scalar
                eng.dma_start(out=t, in_=g_d[P * f : P * (f + 1), :])
                gt.append(t)
            xt = []
            for f in range(NF):
                t = consts.tile([P, M], f32r, name=f"xt{f}", tag=f"xt{f}")
                eng = nc.sync if f == 0 else nc.scalar
                eng.dma_start(out=t, in_=xT_d[P * f : P * (f + 1), :])
                xt.append(t)
            NE = 4  # quarter-chunks of the key stream, [128, 1024] each
            xkh = []
            for f in range(NF):
                pieces = []
                for qq in range(NE):
                    t = io.tile(
                        [P, T // NE], f32r, name=f"xkh{f}_{qq}", tag=f"xk{f}_{qq}"
                    )
                    eng = nc.sync if f == 0 else nc.scalar
                    eng.dma_start(
                        out=t,
                        in_=xkT_d[
                            P * f : P * (f + 1), (T // NE) * qq : (T // NE) * (qq + 1)
                        ],
                    )
                    pieces.append(t)
                xkh.append(pieces)
            # vcomb in 4 group-DMAs: group 0 early on the gpsimd ring; groups
            # 1-3 ride the sync/scalar HWDGE rings BEHIND the xk quarters (ring
            # FIFO order keeps them off the critical early HBM window)
            vcall = store.tile([P, NJ * NVC], bf16, name="vcall", tag="vc")
            GW = 8 * NVC
            for g, eng in [(0, nc.gpsimd), (1, nc.sync), (2, nc.scalar), (3, nc.sync)]:
                eng.dma_start(
                    out=vcall[:, GW * g : GW * (g + 1)],
                    in_=vc_d[:, GW * g : GW * (g + 1)],
                )
            vc = [vcall[:, NVC * j : NVC * (j + 1)] for j in range(NJ)]
            aux = consts.tile([P, 8 * P + NS * TAIL], bf16, name="aux")
            nc.gpsimd.dma_start(out=aux, in_=aux_d[:, :])
            bias_t = consts.tile([P, 1], f32, name="bias_t")
            nc.vector.memset(bias_t, -CSUB)

            # ---- zT = G @ x^T  (the q-side projection through G) ----
            zt = []
            for h in range(NH):
                ps = psA.tile([P, M], f32, name=f"zps{h}", tag="ps")
                for f in range(NF):
                    nc.tensor.matmul(
                        ps,
                        gt[f][:, P * h : P * (h + 1)],
                        xt[f],
                        start=(f == 0),
                        stop=(f == NF - 1),
                    )
                t = store.tile([P, M], f32r, name=f"zt{h}", tag=f"zt{h}")
                if h == 0:
                    nc.vector.tensor_copy(t, ps)
                else:
                    nc.scalar.copy(t, ps)
                zt.append(t)

            # ---- pipeline: S^T(j) = x_keyT-tile^T @ zT directly from the DMA
            # quarters; exp; Ru lagging DELAY j-tiles ----
            ru = []
            for c in range(NMAIN):
                ru.append(psR.tile([P, NVC], f32, name=f"ru{c}", tag="ru"))
            elist = []
            DELAY = 2

            def ru_step(j):
                for c in range(NMAIN):
                    nc.tensor.matmul(
                        ru[c],
                        elist[j][:, P * c : P * (c + 1)],
                        vc[j],
                        start=(j == 0),
                        stop=(j == NJ - 1),
                    )

            for j in range(NJ):
                qq, jq = divmod(j, 8)
                ps = psA.tile([P, M], f32, name=f"sps{j}", tag="ps")
                for f in range(NF):
                    nc.tensor.matmul(
                        ps,
                        xkh[f][qq][:, P * jq : P * (jq + 1)],
                        zt[f],
                        start=(f == 0),
                        stop=(f == NF - 1),
                    )
                ej = store.tile([P, M], bf16, name=f"e{j}", tag="E", bufs=NJ)
                nc.scalar.activation(ej, ps, Exp, bias=bias_t)
                elist.append(ej)
                if j >= DELAY:
                    ru_step(j - DELAY)
            for j in range(NJ - DELAY, NJ):
                ru_step(j)

            # ---- normalize + combine, chunk by chunk ----
            # aux layout (bf16), all [128, 128] banded matrices with w_s baked:
            #   A1 = aux[:, 128s:128(s+1)]        k == p - 2s       (own chunk)
            #   A2 = aux[:, 512+128s:512+128(s+1)] k == 128 + p - 2s (prev chunk)
            #   A3 = aux[:, 1024+6s:1024+6(s+1)]  k == 128 + t' - 2s (tail rows)
            rch = []
            for c in range(NMAIN):
                rec = small.tile([P, 1], f32, name=f"rec{c}", tag="rec")
                nc.vector.reciprocal(rec, ru[c][:, NS * NV : NVC])
                t = store.tile([P, NS * NV], bf16, name=f"r{c}", tag=f"r{c}")
                nc.vector.tensor_scalar_mul(t, ru[c][:, 0 : NS * NV], rec)
                rch.append(t)

            oall = small.tile([P, NMAIN, NV], f32, name="oall", tag="osb")
            for c in range(NMAIN):
                po = psA.tile([P, NV], f32, name=f"po{c}", tag="ps")
                nmm = NS + (NS - 1 if c > 0 else 0)
                i = 0
                for s in range(NS):
                    nc.tensor.matmul(
                        po,
                        aux[:, P * s : P * (s + 1)],
                        rch[c][:, NV * s : NV * (s + 1)],
                        start=(i == 0),
                        stop=(i == nmm - 1),
                    )
                    i += 1
                if c > 0:
                    for s in range(1, NS):
                        nc.tensor.matmul(
                            po,
                            aux[:, 4 * P + P * s : 4 * P + P * (s + 1)],
                            rch[c - 1][:, NV * s : NV * (s + 1)],
                            start=False,
                            stop=(i == nmm - 1),
                        )
                        i += 1
                nc.vector.tensor_copy(oall[:, c, :], po)
            nc.sync.dma_start(out=out_d.rearrange("c p v -> p c v"), in_=oall)

            # tail rows [512, 518): next core's missing contribution
            pot = psA.tile([TAIL, NV], f32, name="pot", tag="ps")
            for s in range(1, NS):
                nc.tensor.matmul(
                    pot,
                    aux[:, 8 * P + TAIL * s : 8 * P + TAIL * (s + 1)],
                    rch[NMAIN - 1][:, NV * s : NV * (s + 1)],
                    start=(s == 1),
                    stop=(s == NS - 1),
                )
            ot = small.tile([TAIL, NV], f32, name="ot", tag="ot")
            nc.vector.tensor_copy(ot, pot)
            nc.sync.dma_start(out=outt_d[:, :], in_=ot)

    nc.compile()
    return nc


def _get_nc():
    if "nc" not in _NC_CACHE:
        _install_axon_ntff_hook()
        _NC_CACHE["nc"] = _build_nc()
    return _NC_CACHE["nc"]


def _host_prep(x, x_key, x_value, W_qk, w_shift):
    bf = ml_dtypes.bfloat16
    x = np.ascontiguousarray(np.asarray(x, dtype=np.float32))
    x_key = np.ascontiguousarray(np.asarray(x_key, dtype=np.float32))
    x_value = np.ascontiguousarray(np.asarray(x_value, dtype=np.float32))
    W_qk = np.ascontiguousarray(np.asarray(W_qk, dtype=np.float32))
    w_shift = np.asarray(w_shift, dtype=np.float32)

    xkT = np.ascontiguousarray(x_key.T)                      # [Q, T]
    gmat = np.ascontiguousarray(
        (W_qk.astype(np.float64).T @ W_qk.astype(np.float64)).astype(np.float32)
    )                                                        # [Q, Q], symmetric

    vcomb = np.zeros((T, NVC), np.float32)
    for s in range(NS):
        d = STEP * s
        vcomb[: T - d, NV * s : NV * (s + 1)] = x_value[d:, :]
    vcomb[:, NS * NV] = 1.0
    # pre-tile: [T, NVC] -> [NJ, P, NVC] -> [P, NJ*NVC] so each SBUF partition
    # line is one contiguous DMA descriptor
    vcomb = np.ascontiguousarray(
        vcomb.astype(bf).reshape(NJ, P, NVC).transpose(1, 0, 2).reshape(P, NJ * NVC)
    )

    # combine matrices (see aux layout comment in _build_nc)
    aux = np.zeros((P, 8 * P + NS * TAIL), np.float32)
    for s in range(NS):
        w = w_shift[0, s]
        for p in range(P):
            k = p - STEP * s
            if 0 <= k < P:
                aux[k, P * s + p] = w                      # A1
            kk = P + p - STEP * s
            if 0 <= kk < P:
                aux[kk, 4 * P + P * s + p] = w             # A2 (prev chunk)
        if s >= 1:
            for tp in range(TAIL):
                k = P + tp - STEP * s
                if 0 <= k < P:
                    aux[k, 8 * P + TAIL * s + tp] = w      # A3 (tail rows)
    aux = aux.astype(bf)

    in_maps = []
    for d in range(NCORES):
        r0 = d * M
        xT = np.ascontiguousarray(x[r0 : r0 + M].T)          # [Q, M]
        in_maps.append(
            {"xT": xT, "xkT": xkT, "gmat": gmat, "vcomb": vcomb, "aux": aux}
        )
    return in_maps


def kernel(x, x_key, x_value, W_qk, w_shift):
    global LAST_RESULT
    from concourse.bass_utils import run_bass_kernel_spmd

    nc = _get_nc()
    in_maps = _host_prep(x, x_key, x_value, W_qk, w_shift)
    res = run_bass_kernel_spmd(nc, in_maps, core_ids=list(range(NCORES)))
    LAST_RESULT = res
    out = np.concatenate(
        [res.results[d]["out"].reshape(M, NV) for d in range(NCORES)], axis=0
    )
    # add the 6-row cross-core overlap contributions
    for d in range(NCORES - 1):
        out[M * (d + 1) : M * (d + 1) + TAIL] += res.results[d]["outt"]
    return out.astype(np.float32)


# revision 32
# speedup vs baseline: 1.0640x; 1.0231x over previous
"""Trainium2 Bass kernel for shifted sparse attention (nn_Attention_74672301408506).

Math (reference):
    q = x @ W.T ; k = x_key @ W.T ; att = softmax(q k^T)
    out[i, v] = sum_s w_s * sum_j att[i-2s, j] * x_value[j+2s, v]

Device algorithm (8 cores, query rows sharded, 512 rows per core, no halo):
    per core, query rows [r0, r0+512):
      qT[h, i], kT[h, j]     (float32r = tf32 matmuls, full-rate at N>=256)
      S^T[j, i] = kT^T q     (scores transposed: keys on partitions)
      E = exp(S - 100)       (bf16; softmax is shift-invariant)
      Ru[i, :] = E^T @ [V_0|V_1|V_2|V_3|ones]   (ones col = softmax denominator)
      R = Ru[:, :320] / Ru[:, 320]
      C[t, v] = sum_s w_s R[t - 2s, 80s + v]  for t in [0, 518)  (banded matmuls)
    C rows [0, 512) are complete except the first 6 rows, which miss the
    neighbor's contribution; rows [512, 518) are exactly that contribution for
    the next core. The host adds the 6-row overlaps when unsharding (exact).
"""

import os
import sys
import types

import numpy as np
import ml_dtypes

T = 4096
Q = 256
H = 256
NV = 80
NS = 4
STEP = 2
NCORES = 8
M = T // NCORES            # 512 rows per core
CSUB = 100.0               # global score shift before exp
P = 128
NJ = T // P                # 32 key tiles
NF = Q // P                # 2 feature tiles
NH = H // P                # 2 hidden tiles
NMAIN = M // P             # 4 i-chunks of 128
NVC = NS * NV + 1          # value width incl ones column
TAIL = (NS - 1) * STEP     # 6 overlap rows between neighboring cores


def _install_axon_ntff_hook():
    """bass_utils' trace path imports antenv.axon_hooks, which the agent image
    lacks; shim it and register the ctypes-based NTFF profiler hook."""
    if "antenv.axon_hooks" in sys.modules:
        return
    try:
        import antenv
    except ImportError:
        return
    mod = types.ModuleType("antenv.axon_hooks")
    mod._hook = None
    mod.set_axon_ntff_profile_hook = lambda h: setattr(mod, "_hook", h)
    mod.get_axon_ntff_profile_hook = lambda: mod._hook
    sys.modules["antenv.axon_hooks"] = mod
    antenv.axon_hooks = mod
    try:
        from trn_agent_boot import trn_boot

        so_path = "/opt/axon/libaxon_pjrt.so"
        if os.path.exists(so_path):
            mod.set_axon_ntff_profile_hook(trn_boot._ntff_profile_via_ctypes(so_path))
    except Exception:
        pass


_NC_CACHE = {}
LAST_RESULT = None


def _build_nc():
    import concourse.mybir as mybir
    import concourse.tile as tile
    from concourse import bacc

    f32 = mybir.dt.float32
    f32r = mybir.dt.float32r
    bf16 = mybir.dt.bfloat16
    Exp = mybir.ActivationFunctionType.Exp

    nc = bacc.Bacc(None, target_bir_lowering=False)

    xT_d = nc.dram_tensor("xT", [Q, M], f32r, kind="ExternalInput")
    xkT_d = nc.dram_tensor("xkT", [Q, T], f32r, kind="ExternalInput")
    # G = W^T W (host-side): scores = x_key G x^T, so kT is never materialized
    g_d = nc.dram_tensor("gmat", [Q, Q], f32r, kind="ExternalInput")
    # pre-tiled on host: row p holds tile-row p of every j-tile (big DMA descs)
    vc_d = nc.dram_tensor("vcomb", [P, NJ * NVC], bf16, kind="ExternalInput")
    aux_d = nc.dram_tensor("aux", [P, 8 * P + NS * TAIL], bf16, kind="ExternalInput")
    out_d = nc.dram_tensor("out", [NMAIN, P, NV], f32, kind="ExternalOutput")
    outt_d = nc.dram_tensor("outt", [TAIL, NV], f32, kind="ExternalOutput")

    with tile.TileContext(nc) as tc:
        with (
            tc.tile_pool(name="consts", bufs=1) as consts,
            tc.tile_pool(name="io", bufs=1) as io,
            tc.tile_pool(name="store", bufs=1) as store,
            tc.tile_pool(name="small", bufs=6) as small,
            tc.tile_pool(name="psA", bufs=4, space="PSUM") as psA,
            tc.tile_pool(name="psR", bufs=4, space="PSUM") as psR,
        ):
            # ---- PE warmup: dummy matmuls while input DMAs stream, so HAM
            # reaches K=8/8 (2.4 GHz) before real work ----
            wu = consts.tile([P, 512], bf16, name="wu")
            nc.vector.memset(wu, 0.0)
            wups = psA.tile([P, 512], f32, name="wups", tag="ps")
            for i in range(14):
                nc.tensor.matmul(wups, wu[:, 0:P], wu, start=True, stop=True)

            # ---- inputs; spread across HWDGE rings (sync+scalar) and SWDGE ----
            gt = []
            for f in range(NF):
                t = consts.tile([P, Q], f32r, name=f"gt{f}", tag=f"gt{f}")
                eng = nc.sync if f == 0 else nc.scalar
                eng.dma_start(out=t, in_=g_d[P * f : P * (f + 1), :])
                gt.append(t)
            xt = []
            for f in range(NF):
                t = consts.tile([P, M], f32r, name=f"xt{f}", tag=f"xt{f}")
                eng = nc.sync if f == 0 else nc.scalar
                eng.dma_start(out=t, in_=xT_d[P * f : P * (f + 1), :])
                xt.append(t)
            NE = 4  # quarter-chunks of the key stream, [128, 1024] each
            xkh = []
            for f in range(NF):
                pieces = []
                for qq in range(NE):
                    t = io.tile(
                        [P, T // NE], f32r, name=f"xkh{f}_{qq}", tag=f"xk{f}_{qq}"
                    )
                    eng = nc.sync if f == 0 else nc.scalar
                    eng.dma_start(
                        out=t,
                        in_=xkT_d[
                            P * f : P * (f + 1), (T // NE) * qq : (T // NE) * (qq + 1)
                        ],
                    )
                    pieces.append(t)
                xkh.append(pieces)
            # vcomb in 4 group-DMAs: group 0 early on the gpsimd ring; groups
            # 1-3 ride the sync/scalar HWDGE rings BEHIND the xk quarters (ring
            # FIFO order keeps them off the critical early HBM window)
            vcall = store.tile([P, NJ * NVC], bf16, name="vcall", tag="vc")
            GW = 8 * NVC
            for g, eng in [(0, nc.gpsimd), (1, nc.sync), (2, nc.scalar), (3, nc.sync)]:
                eng.dma_start(
                    out=vcall[:, GW * g : GW * (g + 1)],
                    in_=vc_d[:, GW * g : GW * (g + 1)],
                )
            vc = [vcall[:, NVC * j : NVC * (j + 1)] for j in range(NJ)]
            aux = consts.tile([P, 8 * P + NS * TAIL], bf16, name="aux")
            nc.gpsimd.dma_start(out=aux, in_=aux_d[:, :])
            bias_t = consts.tile([P, 1], f32, name="bias_t")
            nc.vector.memset(bias_t, -CSUB)

            # ---- zT = G @ x^T  (the q-side projection through G) ----
            zt = []
            for h in range(NH):
                ps = psA.tile([P, M], f32, name=f"zps{h}", tag="ps")
                for f in range(NF):
                    nc.tensor.matmul(
                        ps,
                        gt[f][:, P * h : P * (h + 1)],
                        xt[f],
                        start=(f == 0),
                        stop=(f == NF - 1),
                    )
                t = store.tile([P, M], f32r, name=f"zt{h}", tag=f"zt{h}")
                if h == 0:
                    nc.vector.tensor_copy(t, ps)
                else:
                    nc.scalar.copy(t, ps)
                zt.append(t)

            # ---- pipeline: S^T(j) = x_keyT-tile^T @ zT directly from the DMA
            # quarters; exp; Ru lagging DELAY j-tiles ----
            ru = []
            for c in range(NMAIN):
                ru.append(psR.tile([P, NVC], f32, name=f"ru{c}", tag="ru"))
            elist = []
            DELAY = 2

            def ru_step(j):
                for c in range(NMAIN):
                    nc.tensor.matmul(
                        ru[c],
                        elist[j][:, P * c : P * (c + 1)],
                        vc[j],
                        start=(j == 0),
                        stop=(j == NJ - 1),
                    )

            for j in range(NJ):
                qq, jq = divmod(j, 8)
                ps = psA.tile([P, M], f32, name=f"sps{j}", tag="ps")
                for f in range(NF):
                    nc.tensor.matmul(
                        ps,
                        xkh[f][qq][:, P * jq : P * (jq + 1)],
                        zt[f],
                        start=(f == 0),
                        stop=(f == NF - 1),
                    )
                ej = store.tile([P, M], bf16, name=f"e{j}", tag="E", bufs=NJ)
                nc.scalar.activation(ej, ps, Exp, bias=bias_t)
                elist.append(ej)
                if j >= DELAY:
                    ru_step(j - DELAY)
            for j in range(NJ - DELAY, NJ):
                ru_step(j)

            # ---- normalize + combine, chunk by chunk ----
            # aux layout (bf16), all [128, 128] banded matrices with w_s baked:
            #   A1 = aux[:, 128s:128(s+1)]        k == p - 2s       (own chunk)
            #   A2 = aux[:, 512+128s:512+128(s+1)] k == 128 + p - 2s (prev chunk)
            #   A3 = aux[:, 1024+6s:1024+6(s+1)]  k == 128 + t' - 2s (tail rows)
            rch = []
            for c in range(NMAIN):
                rec = small.tile([P, 1], f32, name=f"rec{c}", tag="rec")
                nc.vector.reciprocal(rec, ru[c][:, NS * NV : NVC])
                t = store.tile([P, NS * NV], bf16, name=f"r{c}", tag=f"r{c}")
                nc.vector.tensor_scalar_mul(t, ru[c][:, 0 : NS * NV], rec)
                rch.append(t)

            oall = small.tile([P, NMAIN, NV], f32, name="oall", tag="osb")
            for c in range(NMAIN):
                po = psA.tile([P, NV], f32, name=f"po{c}", tag="ps")
                nmm = NS + (NS - 1 if c > 0 else 0)
                i = 0
                for s in range(NS):
                    nc.tensor.matmul(
                        po,
                        aux[:, P * s : P * (s + 1)],
                        rch[c][:, NV * s : NV * (s + 1)],
                        start=(i == 0),
                        stop=(i == nmm - 1),
                    )
                    i += 1
                if c > 0:
                    for s in range(1, NS):
                        nc.tensor.matmul(
                            po,
                            aux[:, 4 * P + P * s : 4 * P + P * (s + 1)],
                            rch[c - 1][:, NV * s : NV * (s + 1)],
                            start=False,
                            stop=(i == nmm - 1),
                        )
                        i += 1
                nc.vector.tensor_copy(oall[:, c, :], po)
            nc.sync.dma_start(out=out_d.rearrange("c p v -> p c v"), in_=oall)

            # tail rows [512, 518): next core's missing contribution
            pot = psA.tile([TAIL, NV], f32, name="pot", tag="ps")
            for s in range(1, NS):
                nc.tensor.matmul(
                    pot,
                    aux[:, 8 * P + TAIL * s : 8 * P + TAIL * (s + 1)],
                    rch[NMAIN - 1][:, NV * s : NV * (s + 1)],
                    start=(s == 1),
                    stop=(s == NS - 1),
                )
            ot = small.tile([TAIL, NV], f32, name="ot", tag="ot")
            nc.vector.tensor_copy(ot, pot)
            nc.sync.dma_start(out=outt_d[:, :], in_=ot)

    nc.compile()
    return nc


def _get_nc():
    if "nc" not in _NC_CACHE:
        _install_axon_ntff_hook()
        _NC_CACHE["nc"] = _build_nc()
    return _NC_CACHE["nc"]


def _host_prep(x, x_key, x_value, W_qk, w_shift):
    bf = ml_dtypes.bfloat16
    x = np.ascontiguousarray(np.asarray(x, dtype=np.float32))
    x_key = np.ascontiguousarray(np.asarray(x_key, dtype=np.float32))
    x_value = np.ascontiguousarray(np.asarray(x_value, dtype=np.float32))
    W_qk = np.ascontiguousarray(np.asarray(W_qk, dtype=np.float32))
    w_shift = np.asarray(w_shift, dtype=np.float32)

    xkT = np.ascontiguousarray(x_key.T)                      # [Q, T]
    gmat = np.ascontiguousarray(
        (W_qk.astype(np.float64).T @ W_qk.astype(np.float64)).astype(np.float32)
    )                                                        # [Q, Q], symmetric

    vcomb = np.zeros((T, NVC), np.float32)
    for s in range(NS):
        d = STEP * s
        vcomb[: T - d, NV * s : NV * (s + 1)] = x_value[d:, :]
    vcomb[:, NS * NV] = 1.0
    # pre-tile: [T, NVC] -> [NJ, P, NVC] -> [P, NJ*NVC] so each SBUF partition
    # line is one contiguous DMA descriptor
    vcomb = np.ascontiguousarray(
        vcomb.astype(bf).reshape(NJ, P, NVC).transpose(1, 0, 2).reshape(P, NJ * NVC)
    )

    # combine matrices (see aux layout comment in _build_nc)
    aux = np.zeros((P, 8 * P + NS * TAIL), np.float32)
    for s in range(NS):
        w = w_shift[0, s]
        for p in range(P):
            k = p - STEP * s
            if 0 <= k < P:
                aux[k, P * s + p] = w                      # A1
            kk = P + p - STEP * s
            if 0 <= kk < P:
                aux[kk, 4 * P + P * s + p] = w             # A2 (prev chunk)
        if s >= 1:
            for tp in range(TAIL):
                k = P + tp - STEP * s
                if 0 <= k < P:
                    aux[k, 8 * P + TAIL * s + tp] = w      # A3 (tail rows)
    aux = aux.astype(bf)

    in_maps = []
    for d in range(NCORES):
        r0 = d * M
        xT = np.ascontiguousarray(x[r0 : r0 + M].T)          # [Q, M]
        in_maps.append(
            {"xT": xT, "xkT": xkT, "gmat": gmat, "vcomb": vcomb, "aux": aux}
        )
    return in_maps


def kernel(x, x_key, x_value, W_qk, w_shift):
    global LAST_RESULT
    from concourse.bass_utils import run_bass_kernel_spmd

    nc = _get_nc()
    in_maps = _host_prep(x, x_key, x_value, W_qk, w_shift)
    res = run_bass_kernel_spmd(nc, in_maps, core_ids=list(range(NCORES)))
    LAST_RESULT = res
    out = np.concatenate(
        [res.results[d]["out"].reshape(M, NV) for d in range(NCORES)], axis=0
    )
    # add the 6-row cross-core overlap contributions
    for d in range(NCORES - 1):
        out[M * (d + 1) : M * (d + 1) + TAIL] += res.results[d]["outt"]
    return out.astype(np.float32)
